# Initial kernel scaffold
#
"""Trainium2 Bass kernel for nn_ConvSparseKernel (sparse-tap conv, 5 taps).

Computation (per reference):
    Wn[k] = row-standardized W[k]  (per (k, out) row: subtract mean over in,
            then L2-normalize)
    y[b, :, oh, ow] = (sum_k Wn[k] @ x[b, :, oh+kh_k, ow+kw_k] + bias) * NF

Shapes (full): x [16, 256, 64, 64] f32, W [5, 256, 256] f32, bias [256] f32
Output: [16, 256, 62, 62] f32.

Sharding: data-parallel over batch — 8 cores x 2 batches each; W/bias
replicated. Everything (standardization included) runs on-device.

Per-core kernel:
  - W loaded first (SWDGE queue, parallel with x loads on HWDGE) as
    [128 part (out-half), 5, 256]; standardized with batched DVE ops;
    PE-transposed into wt[oc] = [128 part (in-sub), 5, 2, 128 (out-sub)]
    as float32r. oc=0 prep emitted first so the matmul stream starts early.
  - x[b, cchunk] loaded as [128 part (in-ch), 64, 64] f32r tiles, split in
    two row-half DMAs for earlier availability.
  - Main loop: for b, oc, row-chunk (8 rows -> N=496): one PSUM bank
    accumulates 10 fp32r matmuls (5 taps x 2 in-chunks); ACT applies
    (acc * NF + bias*NF) and writes SBUF; DMA out.
"""

import os

import numpy as np

KVARIANT = os.environ.get("KVARIANT", "swy")

KERNEL_KEYS = ((0, 0), (0, 2), (1, 1), (2, 0), (2, 2))
IN_CH = 256
OUT_CH = 256
H = 64
OH = 62
B_FULL = 16
N_CORES = 8
B_LOCAL = B_FULL // N_CORES
NF = float(1.0 / np.sqrt(IN_CH * len(KERNEL_KEYS) + 1))
ROW_CHUNK = 8  # rows of output per PSUM tile -> N = 8*62 = 496 <= 512

_compiled_nc = None


def _emit(tc, nc, y, x, w, bias, conv_repeat=1):
    import concourse.mybir as mybir
    from concourse.masks import make_identity

    f32 = mybir.dt.float32
    f32r = mybir.dt.float32r
    AF = mybir.ActivationFunctionType
    AX = mybir.AxisListType
    NTAP = len(KERNEL_KEYS)

    with tc.tile_pool(name="const", bufs=1) as cpool:
        # gpsimd memset/affine_select can't write f32r tiles; build the
        # identity in f32 and round-copy to f32r via DVE (matmul requires
        # both operands f32r when either is).
        ident_f32 = cpool.tile([128, 128], f32, name="ident_f32")
        make_identity(nc, ident_f32)
        ident = cpool.tile([128, 128], f32r, name="ident")
        nc.vector.tensor_copy(out=ident, in_=ident_f32)
        # Preload the Sqrt ACT table so it isn't loaded lazily on the
        # critical weight-prep chain.
        sqrt_warm = cpool.tile([128, 1], f32, name="sqrt_warm")
        nc.scalar.sqrt(sqrt_warm, ident_f32[:, 0:1])

        # ---- W (oc halves) + bias first on the sync DMA queue; weight
        # prep is the longest startup chain so its data must land first.
        w_okI = w.rearrange("k o i -> o k i")
        bias2d = bias.rearrange("(p u) -> p u", u=1)
        wraw = [cpool.tile([128, NTAP, IN_CH], f32, name=f"wraw_{oc}",
                           tag=f"wraw_{oc}") for oc in range(2)]
        braw = [cpool.tile([128, 1], f32, name=f"braw_{oc}",
                           tag=f"braw_{oc}") for oc in range(2)]
        xt = [[cpool.tile([128, H, H], f32r, name=f"xt_{b}_{cc}",
                          tag=f"xt_{b}_{cc}") for cc in range(2)]
              for b in range(B_LOCAL)]

        # W oc0 per-tap so the per-tap stats chain starts on tap 0's
        # landing; first x quarters interleaved so the early conv chunks
        # unblock close behind the weights.
        # x transfers: 32-row halves (1MB, 8KB/partition) alternating
        # between the SP and ACT HWDGE queues; y-outs are per-chunk
        # (~0.25MB) also alternating. Measured ~1.1TB/s aggregate vs
        # 153GB/s for 2MB transfers on a single queue.
        def xh(b, cc, hh, eng):
            r = slice(hh * 32, (hh + 1) * 32)
            eng.dma_start(out=xt[b][cc][:, r, :],
                          in_=x[b, cc * 128:(cc + 1) * 128, r, :])

        def issue_input_dmas():
            for k in range(3):
                nc.sync.dma_start(out=wraw[0][:, k, :], in_=w_okI[0:128, k, :])
            xh(0, 0, 0, nc.sync if KVARIANT in ("allsp", "swy", "mixy") else nc.scalar)
            nc.sync.dma_start(out=wraw[0][:, 3, :], in_=w_okI[0:128, 3, :])
            xh(0, 1, 0, nc.sync if KVARIANT in ("allsp", "swy", "mixy") else nc.scalar)
            nc.sync.dma_start(out=wraw[0][:, 4, :], in_=w_okI[0:128, 4, :])
            nc.sync.dma_start(out=braw[0], in_=bias2d[0:128])
            xh(0, 0, 1, nc.sync)
            xh(0, 1, 1, nc.sync if KVARIANT in ("allsp", "swy", "mixy") else nc.scalar)
            nc.sync.dma_start(out=wraw[1], in_=w_okI[128:256])
            (nc.sync if KVARIANT in ("allsp", "swy", "mixy") else nc.scalar).dma_start(
                out=braw[1], in_=bias2d[128:256])
            qi = 0
            for b in range(1, B_LOCAL):
                for cc in range(2):
                    for hh in range(2):
                        if KVARIANT in ("allsp", "swy", "mixy"):
                            eng = nc.sync
                        else:
                            eng = nc.sync if qi % 2 == 0 else nc.scalar
                        xh(b, cc, hh, eng)
                        qi += 1
            for oc in range(2):
                nc.scalar.mul(bnf[oc], braw[oc], NF)

        bnf = [cpool.tile([128, 1], f32, name=f"bnf_{oc}", tag=f"bnf_{oc}")
               for oc in range(2)]

        # ---- weight standardization (batched) + PE transpose ----
        # ||w - mu||^2 = ssq - mu*sums, so sq/ssq don't wait on the mean.
        # wt[oc][:, k, ic, :] = [128 (in-sub), 128 (out-sub)] f32r lhsT tile
        wt = [cpool.tile([128, NTAP, 2, 128], f32r, name=f"wt_{oc}",
                         tag=f"wt_{oc}") for oc in range(2)]

        with tc.tile_pool(name="wprep", bufs=2) as wpool, \
             tc.tile_pool(name="tpsum", bufs=3, space="PSUM") as tpool, \
             tc.tile_pool(name="mmpsum", bufs=5, space="PSUM") as mpool, \
             tc.tile_pool(name="outp", bufs=int(os.environ.get("OBUFS", "12"))) as opool:

            def stats_tap(oc, wn, st, k):
                """Per-tap standardization chain: st holds [128, NTAP] stat
                tiles (ssq, sums, mu, musums, var, sd, inv) + sq scratch."""
                ks = slice(k, k + 1)
                # ssq_k = sum(w_k^2) on ACT (Square + accum), off the DVE
                # chain. (tensor_tensor_reduce wedges TRN2 here.)
                nc.scalar.activation(st["sqscratch"], wraw[oc][:, k, :],
                                     AF.Square, accum_out=st["ssq"][:, ks])
                nc.vector.reduce_sum(out=st["sums"][:, ks],
                                     in_=wraw[oc][:, k, :], axis=AX.X)
                nc.vector.tensor_scalar_mul(st["mu"][:, ks],
                                            st["sums"][:, ks], 1.0 / IN_CH)
                nc.vector.tensor_mul(out=st["musums"][:, ks],
                                     in0=st["mu"][:, ks],
                                     in1=st["sums"][:, ks])
                nc.vector.tensor_sub(out=st["var"][:, ks],
                                     in0=st["ssq"][:, ks],
                                     in1=st["musums"][:, ks])
                nc.scalar.sqrt(st["sd"][:, ks], st["var"][:, ks])
                nc.vector.reciprocal(st["inv"][:, ks], st["sd"][:, ks])
                # wn_k = (w_k - mu_k) * inv_k, one fused DVE op
                nc.vector.tensor_scalar(
                    out=wn[:, k, :], in0=wraw[oc][:, k, :],
                    scalar1=st["mu"][:, ks], scalar2=st["inv"][:, ks],
                    op0=mybir.AluOpType.subtract,
                    op1=mybir.AluOpType.mult)

            def transpose_tap(oc, wn, k):
                for ic in range(2):
                    pt = tpool.tile([128, 128], f32r, name="pt")
                    nc.tensor.transpose(
                        pt, wn[:, k, ic * 128:(ic + 1) * 128], ident)
                    # alternate PSUM->SBUF copy engine: DVE / ACT
                    if ic == 0:
                        nc.vector.tensor_copy(out=wt[oc][:, k, ic, :], in_=pt)
                    else:
                        nc.scalar.copy(wt[oc][:, k, ic, :], pt)

            def alloc_stats(suffix):
                st = {}
                for nm in ("ssq", "sums", "mu", "musums", "var", "sd", "inv"):
                    st[nm] = wpool.tile([128, NTAP], f32, name=f"{nm}{suffix}",
                                        tag=f"{nm}{suffix}")
                st["sqscratch"] = wpool.tile([128, IN_CH], f32,
                                             name=f"sqs{suffix}",
                                             tag=f"sqs{suffix}")
                return st

            def prep(oc):
                st = alloc_stats(f"_{oc}")
                wn = wpool.tile([128, NTAP, IN_CH], f32r, name=f"wn_{oc}",
                                tag=f"wn_{oc}")
                for k in range(NTAP):
                    stats_tap(oc, wn, st, k)
                    transpose_tap(oc, wn, k)

            drain_qi = [0]

            def drain_chunk(b, oc, r0, nr, ps):
                ot = opool.tile([128, nr, OH], f32, name="ot", tag="ot")
                nc.scalar.activation(ot, ps, AF.Identity,
                                     bias=bnf[oc], scale=NF)
                if KVARIANT == "allsp":
                    eng = nc.sync
                elif KVARIANT == "swy":
                    eng = nc.gpsimd
                elif KVARIANT == "mixy":
                    eng = nc.sync if drain_qi[0] % 2 == 0 else nc.gpsimd
                elif KVARIANT == "drainact":
                    eng = nc.scalar
                else:
                    eng = nc.sync if drain_qi[0] % 2 == 0 else nc.scalar
                drain_qi[0] += 1
                eng.dma_start(
                    out=y[b, oc * 128:(oc + 1) * 128, r0:r0 + nr, :], in_=ot)

            def conv_chunk(b, oc, r0):
                nr = min(ROW_CHUNK, OH - r0)
                ps = mpool.tile([128, nr, OH], f32, name="ps", tag="ps")
                idx = 0
                for k, (kh, kw) in enumerate(KERNEL_KEYS):
                    for ic in range(2):
                        rhs = xt[b][ic][:, kh + r0:kh + r0 + nr, kw:kw + OH]
                        nc.tensor.matmul(ps, wt[oc][:, k, ic, :], rhs,
                                         start=(idx == 0), stop=(idx == 9))
                        idx += 1
                drain_chunk(b, oc, r0, nr, ps)

            def conv_group(b, oc, r0_start=0):
                for r0 in range(r0_start, OH, ROW_CHUNK):
                    conv_chunk(b, oc, r0)

            def prewarm_pe(n=22):
                """Dummy transposes while PE would be idle: warms the HAM
                clock gate (~3.4us of activity) so the real matmul stream
                starts at 2.4 GHz."""
                for _ in range(n):
                    ptw = tpool.tile([128, 128], f32r, name="ptw", tag="pt")
                    nc.tensor.transpose(ptw, ident, ident)

            def prep0_fused():
                """oc0 prep with the first NPS chunks' matmuls interleaved
                per tap, so PE work starts as soon as tap 0 is ready."""
                NPS = 3
                st = alloc_stats("_0")
                wn = wpool.tile([128, NTAP, IN_CH], f32r, name="wn_0",
                                tag="wn_0")
                psf = [mpool.tile([128, ROW_CHUNK, OH], f32,
                                  name=f"psf{c}", tag="ps")
                       for c in range(NPS)]
                for k, (kh, kw) in enumerate(KERNEL_KEYS):
                    stats_tap(0, wn, st, k)
                    transpose_tap(0, wn, k)
                    for c in range(NPS):
                        for ic in range(2):
                            r0 = c * ROW_CHUNK
                            rhs = xt[0][ic][:, kh + r0:kh + r0 + ROW_CHUNK,
                                            kw:kw + OH]
                            nc.tensor.matmul(
                                psf[c], wt[0][:, k, ic, :], rhs,
                                start=(k == 0 and ic == 0),
                                stop=(k == NTAP - 1 and ic == 1))
                for c in range(NPS):
                    drain_chunk(0, 0, c * ROW_CHUNK, ROW_CHUNK, psf[c])
                return NPS * ROW_CHUNK

            def whole_kernel():
                issue_input_dmas()
                r0_rest = prep0_fused()
                conv_group(0, 0, r0_rest)
                # oc1 prep overlaps the conv matmul stream
                prep(1)
                conv_group(0, 1)
                for b in range(1, B_LOCAL):
                    conv_group(b, 0)
                    conv_group(b, 1)

            if conv_repeat == 1:
                prewarm_pe()
                whole_kernel()
            else:
                # timing variant: loop the ENTIRE kernel (DMAs + prep +
                # conv) to measure full steady-state per-kernel time
                import concourse.mybir as _mb
                with tc.For_i(0, conv_repeat, 1,
                              hint_engines=(_mb.EngineType.PE,)) as _i:
                    whole_kernel()


def _build_nc(conv_repeat=1):
    import concourse.mybir as mybir
    import concourse.tile as tile
    from concourse import bacc

    f32 = mybir.dt.float32
    f32r = mybir.dt.float32r
    nc = bacc.Bacc("TRN2", target_bir_lowering=False, debug=False)
    x = nc.dram_tensor("x", (B_LOCAL, IN_CH, H, H), f32r,
                       kind="ExternalInput").ap()
    w = nc.dram_tensor("w", (len(KERNEL_KEYS), OUT_CH, IN_CH), f32,
                       kind="ExternalInput").ap()
    bias = nc.dram_tensor("bias", (OUT_CH,), f32, kind="ExternalInput").ap()
    y = nc.dram_tensor("y", (B_LOCAL, OUT_CH, OH, OH), f32,
                       kind="ExternalOutput").ap()

    with tile.TileContext(nc) as tc:
        _emit(tc, nc, y, x, w, bias, conv_repeat=conv_repeat)
    nc.compile()
    return nc


def _get_nc():
    global _compiled_nc
    if _compiled_nc is None:
        _compiled_nc = _build_nc()
    return _compiled_nc


def _make_in_maps(x, W, bias):
    x = np.ascontiguousarray(x, dtype=np.float32)
    W = np.ascontiguousarray(W, dtype=np.float32)
    bias = np.ascontiguousarray(bias, dtype=np.float32)
    return [
        {
            "x": np.ascontiguousarray(x[i * B_LOCAL:(i + 1) * B_LOCAL]),
            "w": W,
            "bias": bias,
        }
        for i in range(N_CORES)
    ]


def kernel(x, W, bias):
    from concourse import bass_utils

    nc = _get_nc()
    res = bass_utils.run_bass_kernel_spmd(
        nc, _make_in_maps(x, W, bias), core_ids=list(range(N_CORES)))
    return np.concatenate([r["y"] for r in res.results], axis=0)



# revision 2
# speedup vs baseline: 1.0036x; 1.0036x over previous
"""Trainium2 Bass kernel for nn_ConvSparseKernel (sparse-tap conv, 5 taps).

Computation (per reference):
    Wn[k] = row-standardized W[k]  (per (k, out) row: subtract mean over in,
            then L2-normalize)
    y[b, :, oh, ow] = (sum_k Wn[k] @ x[b, :, oh+kh_k, ow+kw_k] + bias) * NF

Shapes (full): x [16, 256, 64, 64] f32, W [5, 256, 256] f32, bias [256] f32
Output: [16, 256, 62, 62] f32.

Sharding: data-parallel over batch — 8 cores x 2 batches each; W/bias
replicated. Everything (standardization included) runs on-device.

Per-core kernel:
  - x[b, cchunk] loaded as [128 part (in-ch), 64, 64] f32r tiles, split in
    two row-half DMAs for earlier availability.
  - W standardized per tap with the work spread across ACT (square/sqrt),
    DVE (reduce/recip/normalize) and Pool (small scalar algebra) so the
    per-tap chain pipelines at < the PE's per-tap matmul time; emission is
    staggered one tap ahead to avoid head-of-line blocking on ACT.
  - PE warms up on f32 identity transposes immediately (no dependency on
    weight data), then tap-k weight transposes + the first conv chunks'
    matmuls are interleaved with the stats stream.
  - Main loop: for b, oc, row-chunk (8 rows -> N=496): one PSUM bank
    accumulates 10 fp32r matmuls (5 taps x 2 in-chunks); ACT applies
    (acc * NF + bias*NF) and writes SBUF; DMA out on the gpsimd queue.
  - The very last chunk's store is split across two DMA queues to shorten
    the drain tail.
"""

import os

import numpy as np

KERNEL_KEYS = ((0, 0), (0, 2), (1, 1), (2, 0), (2, 2))
IN_CH = 256
OUT_CH = 256
H = 64
OH = 62
B_FULL = 16
N_CORES = 8
B_LOCAL = B_FULL // N_CORES
NF = float(1.0 / np.sqrt(IN_CH * len(KERNEL_KEYS) + 1))
ROW_CHUNK = 8  # rows of output per PSUM tile -> N = 8*62 = 496 <= 512
PREWARM = int(os.environ.get("PREWARM", "8"))
NPS = int(os.environ.get("NPS", "3"))
OBUFS = int(os.environ.get("OBUFS", "12"))

_compiled_nc = None


def _emit(tc, nc, y, x, w, bias):
    import concourse.mybir as mybir
    from concourse.masks import make_identity

    f32 = mybir.dt.float32
    f32r = mybir.dt.float32r
    AF = mybir.ActivationFunctionType
    AX = mybir.AxisListType
    NTAP = len(KERNEL_KEYS)

    with tc.tile_pool(name="const", bufs=1) as cpool:
        ident_f32 = cpool.tile([128, 128], f32, name="ident_f32")
        make_identity(nc, ident_f32)
        # One ACT op whose table set (sqrt_and_others) also covers Square /
        # Identity / Copy, so no further table loads land on the critical
        # stats chain.
        sqrt_warm = cpool.tile([128, 1], f32, name="sqrt_warm")
        nc.scalar.sqrt(sqrt_warm, ident_f32[:, 0:1])

        # ---- W (oc halves) + bias first on the sync DMA queue; weight
        # prep is the longest startup chain so its data must land first.
        w_okI = w.rearrange("k o i -> o k i")
        bias2d = bias.rearrange("(p u) -> p u", u=1)
        wraw = [cpool.tile([128, NTAP, IN_CH], f32, name=f"wraw_{oc}",
                           tag=f"wraw_{oc}") for oc in range(2)]
        braw = [cpool.tile([128, 1], f32, name=f"braw_{oc}",
                           tag=f"braw_{oc}") for oc in range(2)]
        bnf = [cpool.tile([128, 1], f32, name=f"bnf_{oc}", tag=f"bnf_{oc}")
               for oc in range(2)]
        xt = [[cpool.tile([128, H, H], f32r, name=f"xt_{b}_{cc}",
                          tag=f"xt_{b}_{cc}") for cc in range(2)]
              for b in range(B_LOCAL)]

        # x transfers: 32-row halves (1MB, 8KB/partition) on the SP HWDGE
        # queue; W oc0 per-tap so the per-tap stats chain starts on tap 0's
        # landing; first x quarters interleaved so the early conv chunks
        # unblock close behind the weights.
        def xh(b, cc, hh):
            r = slice(hh * 32, (hh + 1) * 32)
            nc.sync.dma_start(out=xt[b][cc][:, r, :],
                              in_=x[b, cc * 128:(cc + 1) * 128, r, :])

        def issue_input_dmas():
            for k in range(3):
                nc.sync.dma_start(out=wraw[0][:, k, :], in_=w_okI[0:128, k, :])
            xh(0, 0, 0)
            nc.sync.dma_start(out=wraw[0][:, 3, :], in_=w_okI[0:128, 3, :])
            xh(0, 1, 0)
            nc.sync.dma_start(out=wraw[0][:, 4, :], in_=w_okI[0:128, 4, :])
            nc.sync.dma_start(out=braw[0], in_=bias2d[0:128])
            xh(0, 0, 1)
            xh(0, 1, 1)
            nc.sync.dma_start(out=wraw[1], in_=w_okI[128:256])
            nc.sync.dma_start(out=braw[1], in_=bias2d[128:256])
            for b in range(1, B_LOCAL):
                for cc in range(2):
                    for hh in range(2):
                        xh(b, cc, hh)

        # ---- weight standardization + PE transpose ----
        # wt[oc][:, k, ic, :] = [128 (in-sub), 128 (out-sub)] f32r lhsT tile
        wt = [cpool.tile([128, NTAP, 2, 128], f32r, name=f"wt_{oc}",
                         tag=f"wt_{oc}") for oc in range(2)]

        with tc.tile_pool(name="wprep", bufs=2) as wpool, \
             tc.tile_pool(name="tpsum", bufs=3, space="PSUM") as tpool, \
             tc.tile_pool(name="mmpsum", bufs=5, space="PSUM") as mpool, \
             tc.tile_pool(name="outp", bufs=OBUFS) as opool:

            def prewarm_pe(n):
                """f32 identity transposes (no data dependency beyond
                make_identity): busy the PE from ~t=0 so the HW clock-ramp
                window elapses before the real matmul stream begins."""
                for _ in range(n):
                    ptw = tpool.tile([128, 128], f32, name="ptw", tag="pt")
                    nc.tensor.transpose(ptw, ident_f32, ident_f32)

            def stats_a(oc, st, k):
                """ssq_k = sum(w_k^2) on ACT (Square + accum); sums_k on
                DVE. Independent of each other and of other taps."""
                ks = slice(k, k + 1)
                nc.scalar.activation(st["sqscratch"], wraw[oc][:, k, :],
                                     AF.Square, accum_out=st["ssq"][:, ks])
                nc.vector.reduce_sum(out=st["sums"][:, ks],
                                     in_=wraw[oc][:, k, :], axis=AX.X)

            def stats_b(oc, wn, st, k):
                """mu/var algebra on Pool (tiny [128,1] ops), sqrt on ACT,
                reciprocal + fused normalize on DVE."""
                ks = slice(k, k + 1)
                nc.gpsimd.tensor_scalar_mul(st["mu"][:, ks],
                                            st["sums"][:, ks], 1.0 / IN_CH)
                nc.gpsimd.tensor_mul(out=st["musums"][:, ks],
                                     in0=st["mu"][:, ks],
                                     in1=st["sums"][:, ks])
                nc.gpsimd.tensor_sub(out=st["var"][:, ks],
                                     in0=st["ssq"][:, ks],
                                     in1=st["musums"][:, ks])
                nc.scalar.sqrt(st["sd"][:, ks], st["var"][:, ks])
                nc.vector.reciprocal(st["inv"][:, ks], st["sd"][:, ks])
                # wn_k = (w_k - mu_k) * inv_k, one fused DVE op
                nc.vector.tensor_scalar(
                    out=wn[:, k, :], in0=wraw[oc][:, k, :],
                    scalar1=st["mu"][:, ks], scalar2=st["inv"][:, ks],
                    op0=mybir.AluOpType.subtract,
                    op1=mybir.AluOpType.mult)

            def transpose_tap(oc, wn, k):
                for ic in range(2):
                    pt = tpool.tile([128, 128], f32r, name="pt")
                    nc.tensor.transpose(
                        pt, wn[:, k, ic * 128:(ic + 1) * 128], ident)
                    # alternate PSUM->SBUF copy engine: DVE / ACT
                    if ic == 0:
                        nc.vector.tensor_copy(out=wt[oc][:, k, ic, :], in_=pt)
                    else:
                        nc.scalar.copy(wt[oc][:, k, ic, :], pt)

            def alloc_stats(suffix):
                st = {}
                for nm in ("ssq", "sums", "mu", "musums", "var", "sd", "inv"):
                    st[nm] = wpool.tile([128, NTAP], f32, name=f"{nm}{suffix}",
                                        tag=f"{nm}{suffix}")
                st["sqscratch"] = wpool.tile([128, IN_CH], f32,
                                             name=f"sqs{suffix}",
                                             tag=f"sqs{suffix}")
                return st

            def drain_chunk(b, oc, r0, nr, ps, split=False):
                ot = opool.tile([128, nr, OH], f32, name="ot", tag="ot")
                nc.scalar.activation(ot, ps, AF.Identity,
                                     bias=bnf[oc], scale=NF)
                if split:
                    # final chunk: halve the store across two queues so the
                    # tail drain isn't one serialized descriptor-gen + copy
                    nc.gpsimd.dma_start(
                        out=y[b, oc * 128:oc * 128 + 64, r0:r0 + nr, :],
                        in_=ot[0:64])
                    nc.sync.dma_start(
                        out=y[b, oc * 128 + 64:(oc + 1) * 128, r0:r0 + nr, :],
                        in_=ot[64:128])
                else:
                    nc.gpsimd.dma_start(
                        out=y[b, oc * 128:(oc + 1) * 128, r0:r0 + nr, :],
                        in_=ot)

            def conv_chunk(b, oc, r0, split=False):
                nr = min(ROW_CHUNK, OH - r0)
                ps = mpool.tile([128, nr, OH], f32, name="ps", tag="ps")
                idx = 0
                for k, (kh, kw) in enumerate(KERNEL_KEYS):
                    for ic in range(2):
                        rhs = xt[b][ic][:, kh + r0:kh + r0 + nr, kw:kw + OH]
                        nc.tensor.matmul(ps, wt[oc][:, k, ic, :], rhs,
                                         start=(idx == 0), stop=(idx == 9))
                        idx += 1
                drain_chunk(b, oc, r0, nr, ps, split=split)

            def conv_group(b, oc, r0_start=0, split_last=False):
                for r0 in range(r0_start, OH, ROW_CHUNK):
                    last = r0 + ROW_CHUNK >= OH
                    conv_chunk(b, oc, r0, split=(split_last and last))

            def prep0_fused():
                """oc0 prep with the first NPS chunks' matmuls interleaved
                per tap, so PE work starts as soon as tap 0 is ready.
                Stats emission staggered one tap ahead so ACT's in-order
                queue never head-of-line blocks the next tap's Square."""
                st = alloc_stats("_0")
                wn = wpool.tile([128, NTAP, IN_CH], f32r, name="wn_0",
                                tag="wn_0")
                psf = [mpool.tile([128, ROW_CHUNK, OH], f32,
                                  name=f"psf{c}", tag="ps")
                       for c in range(NPS)]
                stats_a(0, st, 0)
                stats_a(0, st, 1)
                for k, (kh, kw) in enumerate(KERNEL_KEYS):
                    stats_b(0, wn, st, k)
                    transpose_tap(0, wn, k)
                    for c in range(NPS):
                        for ic in range(2):
                            r0 = c * ROW_CHUNK
                            rhs = xt[0][ic][:, kh + r0:kh + r0 + ROW_CHUNK,
                                            kw:kw + OH]
                            nc.tensor.matmul(
                                psf[c], wt[0][:, k, ic, :], rhs,
                                start=(k == 0 and ic == 0),
                                stop=(k == NTAP - 1 and ic == 1))
                    if k + 2 < NTAP:
                        stats_a(0, st, k + 2)
                for c in range(NPS):
                    drain_chunk(0, 0, c * ROW_CHUNK, ROW_CHUNK, psf[c])
                return NPS * ROW_CHUNK

            def prep(oc):
                st = alloc_stats(f"_{oc}")
                wn = wpool.tile([128, NTAP, IN_CH], f32r, name=f"wn_{oc}",
                                tag=f"wn_{oc}")
                stats_a(oc, st, 0)
                stats_a(oc, st, 1)
                for k in range(NTAP):
                    stats_b(oc, wn, st, k)
                    transpose_tap(oc, wn, k)
                    if k + 2 < NTAP:
                        stats_a(oc, st, k + 2)

            issue_input_dmas()
            prewarm_pe(PREWARM)
            # f32r identity for the real weight transposes; DVE is idle
            # during the prewarm so this never delays the stats chain.
            ident = cpool.tile([128, 128], f32r, name="ident")
            nc.vector.tensor_copy(out=ident, in_=ident_f32)
            # bias * NF on Pool so the ACT queue head stays free for stats
            for oc in range(2):
                nc.gpsimd.tensor_scalar_mul(bnf[oc], braw[oc], NF)
            r0_rest = prep0_fused()
            conv_group(0, 0, r0_rest)
            # oc1 prep overlaps the conv matmul stream
            prep(1)
            conv_group(0, 1)
            for b in range(1, B_LOCAL):
                conv_group(b, 0)
                conv_group(b, 1, split_last=(b == B_LOCAL - 1))


def _build_nc():
    import concourse.mybir as mybir
    import concourse.tile as tile
    from concourse import bacc

    f32 = mybir.dt.float32
    f32r = mybir.dt.float32r
    nc = bacc.Bacc("TRN2", target_bir_lowering=False, debug=False)
    x = nc.dram_tensor("x", (B_LOCAL, IN_CH, H, H), f32r,
                       kind="ExternalInput").ap()
    w = nc.dram_tensor("w", (len(KERNEL_KEYS), OUT_CH, IN_CH), f32,
                       kind="ExternalInput").ap()
    bias = nc.dram_tensor("bias", (OUT_CH,), f32, kind="ExternalInput").ap()
    y = nc.dram_tensor("y", (B_LOCAL, OUT_CH, OH, OH), f32,
                       kind="ExternalOutput").ap()

    with tile.TileContext(nc) as tc:
        _emit(tc, nc, y, x, w, bias)
    nc.compile()
    return nc


def _get_nc():
    global _compiled_nc
    if _compiled_nc is None:
        _compiled_nc = _build_nc()
    return _compiled_nc


def _make_in_maps(x, W, bias):
    x = np.ascontiguousarray(x, dtype=np.float32)
    W = np.ascontiguousarray(W, dtype=np.float32)
    bias = np.ascontiguousarray(bias, dtype=np.float32)
    return [
        {
            "x": np.ascontiguousarray(x[i * B_LOCAL:(i + 1) * B_LOCAL]),
            "w": W,
            "bias": bias,
        }
        for i in range(N_CORES)
    ]


def kernel(x, W, bias):
    from concourse import bass_utils

    nc = _get_nc()
    res = bass_utils.run_bass_kernel_spmd(
        nc, _make_in_maps(x, W, bias), core_ids=list(range(N_CORES)))
    return np.concatenate([r["y"] for r in res.results], axis=0)


# revision 5
# speedup vs baseline: 1.0214x; 1.0178x over previous
"""Trainium2 Bass kernel for nn_ConvSparseKernel (sparse-tap conv, 5 taps).

Computation (per reference):
    Wn[k] = row-standardized W[k]  (per (k, out) row: subtract mean over in,
            then L2-normalize)
    y[b, :, oh, ow] = (sum_k Wn[k] @ x[b, :, oh+kh_k, ow+kw_k] + bias) * NF

Shapes (full): x [16, 256, 64, 64] f32, W [5, 256, 256] f32, bias [256] f32
Output: [16, 256, 62, 62] f32.

Sharding: data-parallel over batch — 8 cores x 2 batches each; W/bias
replicated. Everything (standardization included) runs on-device.

Per-core kernel:
  - x[b, cchunk] loaded as [128 part (in-ch), 64, 64] f32r tiles, split in
    two row-half DMAs for earlier availability.
  - W standardized per tap with the work spread across ACT (square/sqrt),
    DVE (reduce/recip/normalize) and Pool (small scalar algebra) so the
    per-tap chain pipelines at < the PE's per-tap matmul time; emission is
    staggered one tap ahead to avoid head-of-line blocking on ACT.
  - PE warms up on f32 identity transposes immediately (no dependency on
    weight data), then tap-k weight transposes + the first conv chunks'
    matmuls are interleaved with the stats stream.
  - Main loop: for b, oc, row-chunk (8 rows -> N=496): one PSUM bank
    accumulates 10 fp32r matmuls (5 taps x 2 in-chunks); ACT applies
    (acc * NF + bias*NF) and writes SBUF; DMA out on the gpsimd queue.
  - The very last chunk's store is split across two DMA queues to shorten
    the drain tail.
"""

import os

import numpy as np

KERNEL_KEYS = ((0, 0), (0, 2), (1, 1), (2, 0), (2, 2))
IN_CH = 256
OUT_CH = 256
H = 64
OH = 62
B_FULL = 16
N_CORES = 8
B_LOCAL = B_FULL // N_CORES
NF = float(1.0 / np.sqrt(IN_CH * len(KERNEL_KEYS) + 1))
ROW_CHUNK = 8  # rows of output per PSUM tile -> N = 8*62 = 496 <= 512
PREWARM = int(os.environ.get("PREWARM", "0"))
OBUFS = int(os.environ.get("OBUFS", "12"))

_compiled_nc = None


def _emit(tc, nc, y, x, w, bias):
    import concourse.mybir as mybir
    from concourse.masks import make_identity

    f32 = mybir.dt.float32
    f32r = mybir.dt.float32r
    AF = mybir.ActivationFunctionType
    AX = mybir.AxisListType
    NTAP = len(KERNEL_KEYS)

    with tc.tile_pool(name="const", bufs=1) as cpool:
        ident_f32 = cpool.tile([128, 128], f32, name="ident_f32")
        make_identity(nc, ident_f32)
        # One ACT op whose table set (sqrt_and_others) also covers Square /
        # Identity / Copy, so no further table loads land on the critical
        # stats chain.
        sqrt_warm = cpool.tile([128, 1], f32, name="sqrt_warm")
        nc.scalar.sqrt(sqrt_warm, ident_f32[:, 0:1])

        # ---- W (oc halves) + bias first on the sync DMA queue; weight
        # prep is the longest startup chain so its data must land first.
        w_okI = w.rearrange("k o i -> o k i")
        bias2d = bias.rearrange("(p u) -> p u", u=1)
        wraw = [cpool.tile([128, NTAP, IN_CH], f32, name=f"wraw_{oc}",
                           tag=f"wraw_{oc}") for oc in range(2)]
        braw = [cpool.tile([128, 1], f32, name=f"braw_{oc}",
                           tag=f"braw_{oc}") for oc in range(2)]
        bnf = [cpool.tile([128, 1], f32, name=f"bnf_{oc}", tag=f"bnf_{oc}")
               for oc in range(2)]
        xt = [[cpool.tile([128, H, H], f32r, name=f"xt_{b}_{cc}",
                          tag=f"xt_{b}_{cc}") for cc in range(2)]
              for b in range(B_LOCAL)]

        # The cost model's DMA engine pool is effectively serial (~360 B/ns)
        # with ~650 ns descriptor-gen per DMA on the issuing queue, so the
        # startup is a sequencing problem: W first (small, feeds the stats
        # chain), then batch-0 x in fine row slices sized so the PE's chunk
        # consumption never outruns the x stream, then the bulk (weights
        # half 2, batch 1) which is needed much later.
        def xs(b, cc, r0, r1):
            r = slice(r0, r1)
            nc.sync.dma_start(out=xt[b][cc][:, r, :],
                              in_=x[b, cc * 128:(cc + 1) * 128, r, :])

        def issue_input_dmas():
            nc.sync.dma_start(out=wraw[0][:, 0:2, :], in_=w_okI[0:128, 0:2, :])
            xs(0, 0, 0, 8)
            xs(0, 1, 0, 8)
            nc.sync.dma_start(out=wraw[0][:, 2:5, :], in_=w_okI[0:128, 2:5, :])
            xs(0, 0, 8, 16)
            xs(0, 1, 8, 16)
            nc.sync.dma_start(out=braw[0], in_=bias2d[0:128])
            for r0 in (16, 32, 48):
                xs(0, 0, r0, r0 + 16)
                xs(0, 1, r0, r0 + 16)
            nc.sync.dma_start(out=wraw[1], in_=w_okI[128:256])
            nc.sync.dma_start(out=braw[1], in_=bias2d[128:256])
            for b in range(1, B_LOCAL):
                for cc in range(2):
                    for hh in range(2):
                        xs(b, cc, hh * 32, (hh + 1) * 32)

        # ---- weight standardization + PE transpose ----
        # wt[oc][:, k, ic, :] = [128 (in-sub), 128 (out-sub)] f32r lhsT tile
        wt = [cpool.tile([128, NTAP, 2, 128], f32r, name=f"wt_{oc}",
                         tag=f"wt_{oc}") for oc in range(2)]

        with tc.tile_pool(name="wprep", bufs=2) as wpool, \
             tc.tile_pool(name="tpsum", bufs=3, space="PSUM") as tpool, \
             tc.tile_pool(name="mmpsum", bufs=5, space="PSUM") as mpool, \
             tc.tile_pool(name="outp", bufs=OBUFS) as opool:

            def prewarm_pe(n):
                """f32 identity transposes (no data dependency beyond
                make_identity): busy the PE from ~t=0 so the HW clock-ramp
                window elapses before the real matmul stream begins."""
                for _ in range(n):
                    ptw = tpool.tile([128, 128], f32, name="ptw", tag="pt")
                    nc.tensor.transpose(ptw, ident_f32, ident_f32)

            def stats_a(oc, st, k):
                """ssq_k = sum(w_k^2) on ACT (Square + accum); sums_k on
                DVE. Independent of each other and of other taps."""
                ks = slice(k, k + 1)
                nc.scalar.activation(st["sqscratch"], wraw[oc][:, k, :],
                                     AF.Square, accum_out=st["ssq"][:, ks])
                nc.vector.reduce_sum(out=st["sums"][:, ks],
                                     in_=wraw[oc][:, k, :], axis=AX.X)

            def stats_b(oc, wn, st, k):
                """mu/var algebra on Pool (tiny [128,1] ops), sqrt on ACT,
                reciprocal + fused normalize on DVE."""
                ks = slice(k, k + 1)
                nc.gpsimd.tensor_scalar_mul(st["mu"][:, ks],
                                            st["sums"][:, ks], 1.0 / IN_CH)
                nc.gpsimd.tensor_mul(out=st["musums"][:, ks],
                                     in0=st["mu"][:, ks],
                                     in1=st["sums"][:, ks])
                nc.gpsimd.tensor_sub(out=st["var"][:, ks],
                                     in0=st["ssq"][:, ks],
                                     in1=st["musums"][:, ks])
                nc.scalar.sqrt(st["sd"][:, ks], st["var"][:, ks])
                nc.vector.reciprocal(st["inv"][:, ks], st["sd"][:, ks])
                # wn_k = (w_k - mu_k) * inv_k, one fused DVE op
                nc.vector.tensor_scalar(
                    out=wn[:, k, :], in0=wraw[oc][:, k, :],
                    scalar1=st["mu"][:, ks], scalar2=st["inv"][:, ks],
                    op0=mybir.AluOpType.subtract,
                    op1=mybir.AluOpType.mult)

            def transpose_tap(oc, wn, k):
                for ic in range(2):
                    pt = tpool.tile([128, 128], f32r, name="pt")
                    nc.tensor.transpose(
                        pt, wn[:, k, ic * 128:(ic + 1) * 128], ident)
                    # alternate PSUM->SBUF copy engine: DVE / ACT
                    if ic == 0:
                        nc.vector.tensor_copy(out=wt[oc][:, k, ic, :], in_=pt)
                    else:
                        nc.scalar.copy(wt[oc][:, k, ic, :], pt)

            def alloc_stats(suffix):
                st = {}
                for nm in ("ssq", "sums", "mu", "musums", "var", "sd", "inv"):
                    st[nm] = wpool.tile([128, NTAP], f32, name=f"{nm}{suffix}",
                                        tag=f"{nm}{suffix}")
                st["sqscratch"] = wpool.tile([128, IN_CH], f32,
                                             name=f"sqs{suffix}",
                                             tag=f"sqs{suffix}")
                return st

            def drain_chunk(b, oc, r0, nr, ps, split=False):
                ot = opool.tile([128, nr, OH], f32, name="ot", tag="ot")
                nc.scalar.activation(ot, ps, AF.Identity,
                                     bias=bnf[oc], scale=NF)
                if split:
                    # final chunk: halve the store across two queues so the
                    # tail drain isn't one serialized descriptor-gen + copy
                    nc.gpsimd.dma_start(
                        out=y[b, oc * 128:oc * 128 + 64, r0:r0 + nr, :],
                        in_=ot[0:64])
                    nc.sync.dma_start(
                        out=y[b, oc * 128 + 64:(oc + 1) * 128, r0:r0 + nr, :],
                        in_=ot[64:128])
                else:
                    nc.gpsimd.dma_start(
                        out=y[b, oc * 128:(oc + 1) * 128, r0:r0 + nr, :],
                        in_=ot)

            def conv_chunk(b, oc, r0, split=False):
                nr = min(ROW_CHUNK, OH - r0)
                ps = mpool.tile([128, nr, OH], f32, name="ps", tag="ps")
                idx = 0
                for k, (kh, kw) in enumerate(KERNEL_KEYS):
                    for ic in range(2):
                        rhs = xt[b][ic][:, kh + r0:kh + r0 + nr, kw:kw + OH]
                        nc.tensor.matmul(ps, wt[oc][:, k, ic, :], rhs,
                                         start=(idx == 0), stop=(idx == 9))
                        idx += 1
                drain_chunk(b, oc, r0, nr, ps, split=split)

            def conv_group(b, oc, r0_start=0, split_last=False):
                for r0 in range(r0_start, OH, ROW_CHUNK):
                    last = r0 + ROW_CHUNK >= OH
                    conv_chunk(b, oc, r0, split=(split_last and last))

            # first two chunks of (b0, oc0): a 6-row chunk needing only x
            # rows 0-7 and an 8-row chunk needing rows 6-15, matching the
            # first two fine-grained x DMAs per cc.
            PREP0 = ((0, 6), (6, ROW_CHUNK))

            def prep0_fused():
                """oc0 prep with the first chunks' matmuls interleaved per
                tap, so PE work starts as soon as tap 0 is ready. Stats
                emission staggered one tap ahead so ACT's in-order queue
                never head-of-line blocks the next tap's Square."""
                st = alloc_stats("_0")
                wn = wpool.tile([128, NTAP, IN_CH], f32r, name="wn_0",
                                tag="wn_0")
                psf = [mpool.tile([128, nr, OH], f32, name=f"psf{c}",
                                  tag="ps")
                       for c, (r0, nr) in enumerate(PREP0)]
                stats_a(0, st, 0)
                stats_a(0, st, 1)
                for k, (kh, kw) in enumerate(KERNEL_KEYS):
                    stats_b(0, wn, st, k)
                    transpose_tap(0, wn, k)
                    for c, (r0, nr) in enumerate(PREP0):
                        for ic in range(2):
                            rhs = xt[0][ic][:, kh + r0:kh + r0 + nr,
                                            kw:kw + OH]
                            nc.tensor.matmul(
                                psf[c], wt[0][:, k, ic, :], rhs,
                                start=(k == 0 and ic == 0),
                                stop=(k == NTAP - 1 and ic == 1))
                    if k + 2 < NTAP:
                        stats_a(0, st, k + 2)
                for c, (r0, nr) in enumerate(PREP0):
                    drain_chunk(0, 0, r0, nr, psf[c])
                return PREP0[-1][0] + PREP0[-1][1]

            def prep(oc):
                st = alloc_stats(f"_{oc}")
                wn = wpool.tile([128, NTAP, IN_CH], f32r, name=f"wn_{oc}",
                                tag=f"wn_{oc}")
                stats_a(oc, st, 0)
                stats_a(oc, st, 1)
                for k in range(NTAP):
                    stats_b(oc, wn, st, k)
                    transpose_tap(oc, wn, k)
                    if k + 2 < NTAP:
                        stats_a(oc, st, k + 2)

            issue_input_dmas()
            prewarm_pe(PREWARM)
            # f32r identity for the real weight transposes; DVE is idle
            # during the prewarm so this never delays the stats chain.
            ident = cpool.tile([128, 128], f32r, name="ident")
            nc.vector.tensor_copy(out=ident, in_=ident_f32)
            # bias * NF on Pool so the ACT queue head stays free for stats
            for oc in range(2):
                nc.gpsimd.tensor_scalar_mul(bnf[oc], braw[oc], NF)
            r0_rest = prep0_fused()
            conv_group(0, 0, r0_rest)
            # oc1 prep overlaps the conv matmul stream
            prep(1)
            conv_group(0, 1)
            for b in range(1, B_LOCAL):
                conv_group(b, 0)
                conv_group(b, 1, split_last=(b == B_LOCAL - 1))


def _build_nc():
    import concourse.mybir as mybir
    import concourse.tile as tile
    from concourse import bacc

    f32 = mybir.dt.float32
    f32r = mybir.dt.float32r
    nc = bacc.Bacc("TRN2", target_bir_lowering=False, debug=False)
    x = nc.dram_tensor("x", (B_LOCAL, IN_CH, H, H), f32r,
                       kind="ExternalInput").ap()
    w = nc.dram_tensor("w", (len(KERNEL_KEYS), OUT_CH, IN_CH), f32,
                       kind="ExternalInput").ap()
    bias = nc.dram_tensor("bias", (OUT_CH,), f32, kind="ExternalInput").ap()
    y = nc.dram_tensor("y", (B_LOCAL, OUT_CH, OH, OH), f32,
                       kind="ExternalOutput").ap()

    with tile.TileContext(nc) as tc:
        _emit(tc, nc, y, x, w, bias)
    nc.compile()
    return nc


def _get_nc():
    global _compiled_nc
    if _compiled_nc is None:
        _compiled_nc = _build_nc()
    return _compiled_nc


def _make_in_maps(x, W, bias):
    x = np.ascontiguousarray(x, dtype=np.float32)
    W = np.ascontiguousarray(W, dtype=np.float32)
    bias = np.ascontiguousarray(bias, dtype=np.float32)
    return [
        {
            "x": np.ascontiguousarray(x[i * B_LOCAL:(i + 1) * B_LOCAL]),
            "w": W,
            "bias": bias,
        }
        for i in range(N_CORES)
    ]


def kernel(x, W, bias):
    from concourse import bass_utils

    nc = _get_nc()
    res = bass_utils.run_bass_kernel_spmd(
        nc, _make_in_maps(x, W, bias), core_ids=list(range(N_CORES)))
    return np.concatenate([r["y"] for r in res.results], axis=0)


# revision 10
# speedup vs baseline: 1.0215x; 1.0001x over previous
"""Trainium2 Bass kernel for nn_ConvSparseKernel (sparse-tap conv, 5 taps).

Computation (per reference):
    Wn[k] = row-standardized W[k]  (per (k, out) row: subtract mean over in,
            then L2-normalize)
    y[b, :, oh, ow] = (sum_k Wn[k] @ x[b, :, oh+kh_k, ow+kw_k] + bias) * NF

Shapes (full): x [16, 256, 64, 64] f32, W [5, 256, 256] f32, bias [256] f32
Output: [16, 256, 62, 62] f32.

Sharding: data-parallel over batch — 8 cores x 2 batches each; W/bias
replicated. Everything (standardization included) runs on-device.

Per-core kernel:
  - x[b, cchunk] loaded as [128 part (in-ch), 64, 64] f32r tiles, split in
    two row-half DMAs for earlier availability.
  - W standardized per tap with the work spread across ACT (square/sqrt),
    DVE (reduce/recip/normalize) and Pool (small scalar algebra) so the
    per-tap chain pipelines at < the PE's per-tap matmul time; emission is
    staggered one tap ahead to avoid head-of-line blocking on ACT.
  - PE warms up on f32 identity transposes immediately (no dependency on
    weight data), then tap-k weight transposes + the first conv chunks'
    matmuls are interleaved with the stats stream.
  - Main loop: for b, oc, row-chunk (8 rows -> N=496): one PSUM bank
    accumulates 10 fp32r matmuls (5 taps x 2 in-chunks); ACT applies
    (acc * NF + bias*NF) and writes SBUF; DMA out on the gpsimd queue.
  - The very last chunk's store is split across two DMA queues to shorten
    the drain tail.
"""

import os

import numpy as np

KERNEL_KEYS = ((0, 0), (0, 2), (1, 1), (2, 0), (2, 2))
IN_CH = 256
OUT_CH = 256
H = 64
OH = 62
B_FULL = 16
N_CORES = 8
B_LOCAL = B_FULL // N_CORES
NF = float(1.0 / np.sqrt(IN_CH * len(KERNEL_KEYS) + 1))
ROW_CHUNK = 8  # rows of output per PSUM tile -> N = 8*62 = 496 <= 512
PREWARM = int(os.environ.get("PREWARM", "0"))
OBUFS = int(os.environ.get("OBUFS", "12"))

_compiled_nc = None


def _emit(tc, nc, y, x, w, bias):
    import concourse.mybir as mybir
    from concourse.masks import make_identity

    f32 = mybir.dt.float32
    f32r = mybir.dt.float32r
    AF = mybir.ActivationFunctionType
    AX = mybir.AxisListType
    NTAP = len(KERNEL_KEYS)

    with tc.tile_pool(name="const", bufs=1) as cpool:
        ident_f32 = cpool.tile([128, 128], f32, name="ident_f32")
        make_identity(nc, ident_f32)
        # One ACT op whose table set (sqrt_and_others) also covers Square /
        # Identity / Copy, so no further table loads land on the critical
        # stats chain.
        sqrt_warm = cpool.tile([128, 1], f32, name="sqrt_warm")
        nc.scalar.sqrt(sqrt_warm, ident_f32[:, 0:1])

        # ---- W (oc halves) + bias first on the sync DMA queue; weight
        # prep is the longest startup chain so its data must land first.
        w_okI = w.rearrange("k o i -> o k i")
        bias2d = bias.rearrange("(p u) -> p u", u=1)
        wraw = [cpool.tile([128, NTAP, IN_CH], f32, name=f"wraw_{oc}",
                           tag=f"wraw_{oc}") for oc in range(2)]
        braw = [cpool.tile([128, 1], f32, name=f"braw_{oc}",
                           tag=f"braw_{oc}") for oc in range(2)]
        bnf = [cpool.tile([128, 1], f32, name=f"bnf_{oc}", tag=f"bnf_{oc}")
               for oc in range(2)]
        xt = [[cpool.tile([128, H, H], f32r, name=f"xt_{b}_{cc}",
                          tag=f"xt_{b}_{cc}") for cc in range(2)]
              for b in range(B_LOCAL)]

        # The cost model's DMA engine pool is effectively serial (~360 B/ns)
        # with ~650 ns descriptor-gen per DMA on the issuing queue, so the
        # startup is a sequencing problem: W first (small, feeds the stats
        # chain), then batch-0 x in fine row slices sized so the PE's chunk
        # consumption never outruns the x stream, then the bulk (weights
        # half 2, batch 1) which is needed much later.
        def xs(b, cc, r0, r1):
            r = slice(r0, r1)
            nc.sync.dma_start(out=xt[b][cc][:, r, :],
                              in_=x[b, cc * 128:(cc + 1) * 128, r, :])

        def issue_input_dmas():
            nc.sync.dma_start(out=wraw[0][:, 0:1, :], in_=w_okI[0:128, 0:1, :])
            nc.sync.dma_start(out=wraw[0][:, 1:2, :], in_=w_okI[0:128, 1:2, :])
            xs(0, 0, 0, 8)
            xs(0, 1, 0, 8)
            nc.sync.dma_start(out=wraw[0][:, 2:5, :], in_=w_okI[0:128, 2:5, :])
            xs(0, 0, 8, 16)
            xs(0, 1, 8, 16)
            nc.sync.dma_start(out=braw[0], in_=bias2d[0:128])
            for r0 in (16, 32, 48):
                xs(0, 0, r0, r0 + 16)
                xs(0, 1, r0, r0 + 16)
            nc.sync.dma_start(out=wraw[1], in_=w_okI[128:256])
            nc.sync.dma_start(out=braw[1], in_=bias2d[128:256])
            for b in range(1, B_LOCAL):
                for cc in range(2):
                    for hh in range(2):
                        xs(b, cc, hh * 32, (hh + 1) * 32)

        # ---- weight standardization + PE transpose ----
        # wt[oc][:, k, ic, :] = [128 (in-sub), 128 (out-sub)] f32r lhsT tile
        wt = [cpool.tile([128, NTAP, 2, 128], f32r, name=f"wt_{oc}",
                         tag=f"wt_{oc}") for oc in range(2)]

        with tc.tile_pool(name="wprep", bufs=2) as wpool, \
             tc.tile_pool(name="tpsum", bufs=3, space="PSUM") as tpool, \
             tc.tile_pool(name="mmpsum", bufs=5, space="PSUM") as mpool, \
             tc.tile_pool(name="outp", bufs=OBUFS) as opool:

            def prewarm_pe(n):
                """f32 identity transposes (no data dependency beyond
                make_identity): busy the PE from ~t=0 so the HW clock-ramp
                window elapses before the real matmul stream begins."""
                for _ in range(n):
                    ptw = tpool.tile([128, 128], f32, name="ptw", tag="pt")
                    nc.tensor.transpose(ptw, ident_f32, ident_f32)

            def stats_a(oc, st, k):
                """ssq_k = sum(w_k^2) on ACT (Square + accum); sums_k on
                DVE. Independent of each other and of other taps."""
                ks = slice(k, k + 1)
                nc.scalar.activation(st["sqscratch"], wraw[oc][:, k, :],
                                     AF.Square, accum_out=st["ssq"][:, ks])
                nc.vector.reduce_sum(out=st["sums"][:, ks],
                                     in_=wraw[oc][:, k, :], axis=AX.X)

            def stats_b(oc, wn, st, k):
                """mu/var algebra on Pool (tiny [128,1] ops, with
                musums = (sums/IN_CH)*sums fused into one op), sqrt on
                ACT, reciprocal + fused normalize on DVE. mu is off the
                critical path (parallel to the var->inv chain)."""
                ks = slice(k, k + 1)
                nc.gpsimd.scalar_tensor_tensor(
                    out=st["musums"][:, ks], in0=st["sums"][:, ks],
                    scalar=1.0 / IN_CH, in1=st["sums"][:, ks],
                    op0=mybir.AluOpType.mult, op1=mybir.AluOpType.mult)
                nc.gpsimd.tensor_scalar_mul(st["mu"][:, ks],
                                            st["sums"][:, ks], 1.0 / IN_CH)
                nc.gpsimd.tensor_sub(out=st["var"][:, ks],
                                     in0=st["ssq"][:, ks],
                                     in1=st["musums"][:, ks])
                nc.scalar.sqrt(st["sd"][:, ks], st["var"][:, ks])
                nc.vector.reciprocal(st["inv"][:, ks], st["sd"][:, ks])
                # wn_k = (w_k - mu_k) * inv_k, one fused DVE op
                nc.vector.tensor_scalar(
                    out=wn[:, k, :], in0=wraw[oc][:, k, :],
                    scalar1=st["mu"][:, ks], scalar2=st["inv"][:, ks],
                    op0=mybir.AluOpType.subtract,
                    op1=mybir.AluOpType.mult)

            def transpose_tap(oc, wn, k):
                for ic in range(2):
                    pt = tpool.tile([128, 128], f32r, name="pt")
                    nc.tensor.transpose(
                        pt, wn[:, k, ic * 128:(ic + 1) * 128], ident)
                    # alternate PSUM->SBUF copy engine: DVE / ACT
                    if ic == 0:
                        nc.vector.tensor_copy(out=wt[oc][:, k, ic, :], in_=pt)
                    else:
                        nc.scalar.copy(wt[oc][:, k, ic, :], pt)

            def alloc_stats(suffix):
                st = {}
                for nm in ("ssq", "sums", "mu", "musums", "var", "sd", "inv"):
                    st[nm] = wpool.tile([128, NTAP], f32, name=f"{nm}{suffix}",
                                        tag=f"{nm}{suffix}")
                st["sqscratch"] = wpool.tile([128, IN_CH], f32,
                                             name=f"sqs{suffix}",
                                             tag=f"sqs{suffix}")
                return st

            def drain_chunk(b, oc, r0, nr, ps, split=False):
                ot = opool.tile([128, nr, OH], f32, name="ot", tag="ot")
                nc.scalar.activation(ot, ps, AF.Identity,
                                     bias=bnf[oc], scale=NF)
                if split:
                    # final chunk: halve the store across two otherwise-idle
                    # queues so the tail is two parallel descriptor-gens and
                    # two short copies instead of one serialized large one
                    nc.sync.dma_start(
                        out=y[b, oc * 128:oc * 128 + 64, r0:r0 + nr, :],
                        in_=ot[0:64])
                    nc.scalar.dma_start(
                        out=y[b, oc * 128 + 64:(oc + 1) * 128, r0:r0 + nr, :],
                        in_=ot[64:128])
                else:
                    nc.gpsimd.dma_start(
                        out=y[b, oc * 128:(oc + 1) * 128, r0:r0 + nr, :],
                        in_=ot)

            def conv_chunk(b, oc, r0, split=False):
                nr = min(ROW_CHUNK, OH - r0)
                ps = mpool.tile([128, nr, OH], f32, name="ps", tag="ps")
                idx = 0
                for k, (kh, kw) in enumerate(KERNEL_KEYS):
                    for ic in range(2):
                        rhs = xt[b][ic][:, kh + r0:kh + r0 + nr, kw:kw + OH]
                        nc.tensor.matmul(ps, wt[oc][:, k, ic, :], rhs,
                                         start=(idx == 0), stop=(idx == 9))
                        idx += 1
                drain_chunk(b, oc, r0, nr, ps, split=split)

            def conv_group(b, oc, r0_start=0, split_last=False,
                           mid_hook=None):
                for r0 in range(r0_start, OH, ROW_CHUNK):
                    last = r0 + ROW_CHUNK >= OH
                    conv_chunk(b, oc, r0, split=(split_last and last))
                    if mid_hook is not None and r0 + ROW_CHUNK > OH // 2:
                        mid_hook()
                        mid_hook = None

            # first two chunks of (b0, oc0): a 6-row chunk needing only x
            # rows 0-7 and an 8-row chunk needing rows 6-15, matching the
            # first two fine-grained x DMAs per cc.
            PREP0 = ((0, 6), (6, ROW_CHUNK))

            def prep0_fused():
                """oc0 prep with the first chunks' matmuls interleaved per
                tap, so PE work starts as soon as tap 0 is ready. Stats
                emission staggered one tap ahead so ACT's in-order queue
                never head-of-line blocks the next tap's Square."""
                st = alloc_stats("_0")
                wn = wpool.tile([128, NTAP, IN_CH], f32r, name="wn_0",
                                tag="wn_0")
                psf = [mpool.tile([128, nr, OH], f32, name=f"psf{c}",
                                  tag="ps")
                       for c, (r0, nr) in enumerate(PREP0)]
                stats_a(0, st, 0)
                stats_a(0, st, 1)
                for k, (kh, kw) in enumerate(KERNEL_KEYS):
                    stats_b(0, wn, st, k)
                    transpose_tap(0, wn, k)
                    for c, (r0, nr) in enumerate(PREP0):
                        for ic in range(2):
                            rhs = xt[0][ic][:, kh + r0:kh + r0 + nr,
                                            kw:kw + OH]
                            nc.tensor.matmul(
                                psf[c], wt[0][:, k, ic, :], rhs,
                                start=(k == 0 and ic == 0),
                                stop=(k == NTAP - 1 and ic == 1))
                    if k + 2 < NTAP:
                        stats_a(0, st, k + 2)
                for c, (r0, nr) in enumerate(PREP0):
                    drain_chunk(0, 0, r0, nr, psf[c])
                return PREP0[-1][0] + PREP0[-1][1]

            def prep(oc):
                st = alloc_stats(f"_{oc}")
                wn = wpool.tile([128, NTAP, IN_CH], f32r, name=f"wn_{oc}",
                                tag=f"wn_{oc}")
                stats_a(oc, st, 0)
                stats_a(oc, st, 1)
                for k in range(NTAP):
                    stats_b(oc, wn, st, k)
                    transpose_tap(oc, wn, k)
                    if k + 2 < NTAP:
                        stats_a(oc, st, k + 2)

            issue_input_dmas()
            prewarm_pe(PREWARM)
            # f32r identity for the real weight transposes; DVE is idle
            # during the prewarm so this never delays the stats chain.
            ident = cpool.tile([128, 128], f32r, name="ident")
            nc.vector.tensor_copy(out=ident, in_=ident_f32)
            # bias * NF on Pool so the ACT queue head stays free for stats
            for oc in range(2):
                nc.gpsimd.tensor_scalar_mul(bnf[oc], braw[oc], NF)
            r0_rest = prep0_fused()
            # oc1 prep emitted mid-group so its ACT stats ops enqueue ahead
            # of the later drain-activations (ACT's queue is in-order)
            conv_group(0, 0, r0_rest, mid_hook=lambda: prep(1))
            conv_group(0, 1)
            for b in range(1, B_LOCAL):
                conv_group(b, 0)
                conv_group(b, 1, split_last=(b == B_LOCAL - 1))


def _build_nc():
    import concourse.mybir as mybir
    import concourse.tile as tile
    from concourse import bacc

    f32 = mybir.dt.float32
    f32r = mybir.dt.float32r
    nc = bacc.Bacc("TRN2", target_bir_lowering=False, debug=False)
    x = nc.dram_tensor("x", (B_LOCAL, IN_CH, H, H), f32r,
                       kind="ExternalInput").ap()
    w = nc.dram_tensor("w", (len(KERNEL_KEYS), OUT_CH, IN_CH), f32,
                       kind="ExternalInput").ap()
    bias = nc.dram_tensor("bias", (OUT_CH,), f32, kind="ExternalInput").ap()
    y = nc.dram_tensor("y", (B_LOCAL, OUT_CH, OH, OH), f32,
                       kind="ExternalOutput").ap()

    with tile.TileContext(nc) as tc:
        _emit(tc, nc, y, x, w, bias)
    nc.compile()
    return nc


def _get_nc():
    global _compiled_nc
    if _compiled_nc is None:
        _compiled_nc = _build_nc()
    return _compiled_nc


def _make_in_maps(x, W, bias):
    x = np.ascontiguousarray(x, dtype=np.float32)
    W = np.ascontiguousarray(W, dtype=np.float32)
    bias = np.ascontiguousarray(bias, dtype=np.float32)
    return [
        {
            "x": np.ascontiguousarray(x[i * B_LOCAL:(i + 1) * B_LOCAL]),
            "w": W,
            "bias": bias,
        }
        for i in range(N_CORES)
    ]


def kernel(x, W, bias):
    from concourse import bass_utils

    nc = _get_nc()
    res = bass_utils.run_bass_kernel_spmd(
        nc, _make_in_maps(x, W, bias), core_ids=list(range(N_CORES)))
    return np.concatenate([r["y"] for r in res.results], axis=0)


# revision 13
# speedup vs baseline: 1.0274x; 1.0057x over previous
"""Trainium2 Bass kernel for nn_ConvSparseKernel (sparse-tap conv, 5 taps).

Computation (per reference):
    Wn[k] = row-standardized W[k]  (per (k, out) row: subtract mean over in,
            then L2-normalize)
    y[b, :, oh, ow] = (sum_k Wn[k] @ x[b, :, oh+kh_k, ow+kw_k] + bias) * NF

Shapes (full): x [16, 256, 64, 64] f32, W [5, 256, 256] f32, bias [256] f32
Output: [16, 256, 62, 62] f32.

Sharding: data-parallel over batch — 8 cores x 2 batches each; W/bias
replicated. Everything (standardization included) runs on-device.

Per-core kernel:
  - x[b, cchunk] loaded as [128 part (in-ch), 64, 64] f32r tiles, split in
    two row-half DMAs for earlier availability.
  - W standardized per tap with the work spread across ACT (square/sqrt),
    DVE (reduce/recip/normalize) and Pool (small scalar algebra) so the
    per-tap chain pipelines at < the PE's per-tap matmul time; emission is
    staggered one tap ahead to avoid head-of-line blocking on ACT.
  - PE warms up on f32 identity transposes immediately (no dependency on
    weight data), then tap-k weight transposes + the first conv chunks'
    matmuls are interleaved with the stats stream.
  - Main loop: for b, oc, row-chunk (8 rows -> N=496): one PSUM bank
    accumulates 10 fp32r matmuls (5 taps x 2 in-chunks); ACT applies
    (acc * NF + bias*NF) and writes SBUF; DMA out on the gpsimd queue.
  - The very last chunk's store is split across two DMA queues to shorten
    the drain tail.
"""

import os

import numpy as np

KERNEL_KEYS = ((0, 0), (0, 2), (1, 1), (2, 0), (2, 2))
IN_CH = 256
OUT_CH = 256
H = 64
OH = 62
B_FULL = 16
N_CORES = 8
B_LOCAL = B_FULL // N_CORES
NF = float(1.0 / np.sqrt(IN_CH * len(KERNEL_KEYS) + 1))
ROW_CHUNK = 8  # rows of output per PSUM tile -> N = 8*62 = 496 <= 512
PREWARM = int(os.environ.get("PREWARM", "0"))
OBUFS = int(os.environ.get("OBUFS", "12"))

_compiled_nc = None


def _emit(tc, nc, y, x, w, bias):
    import concourse.mybir as mybir
    from concourse.masks import make_identity

    f32 = mybir.dt.float32
    f32r = mybir.dt.float32r
    AF = mybir.ActivationFunctionType
    AX = mybir.AxisListType
    NTAP = len(KERNEL_KEYS)

    with tc.tile_pool(name="const", bufs=1) as cpool:
        ident_f32 = cpool.tile([128, 128], f32, name="ident_f32")
        make_identity(nc, ident_f32)
        # One ACT op whose table set (sqrt_and_others) also covers Square /
        # Identity / Copy, so no further table loads land on the critical
        # stats chain.
        sqrt_warm = cpool.tile([128, 1], f32, name="sqrt_warm")
        nc.scalar.sqrt(sqrt_warm, ident_f32[:, 0:1])

        # ---- W (oc halves) + bias first on the sync DMA queue; weight
        # prep is the longest startup chain so its data must land first.
        w_okI = w.rearrange("k o i -> o k i")
        bias2d = bias.rearrange("(p u) -> p u", u=1)
        wraw = [cpool.tile([128, NTAP, IN_CH], f32, name=f"wraw_{oc}",
                           tag=f"wraw_{oc}") for oc in range(2)]
        braw = [cpool.tile([128, 1], f32, name=f"braw_{oc}",
                           tag=f"braw_{oc}") for oc in range(2)]
        bnf = [cpool.tile([128, 1], f32, name=f"bnf_{oc}", tag=f"bnf_{oc}")
               for oc in range(2)]
        xt = [[cpool.tile([128, H, H], f32r, name=f"xt_{b}_{cc}",
                          tag=f"xt_{b}_{cc}") for cc in range(2)]
              for b in range(B_LOCAL)]

        # The cost model's DMA engine pool is effectively serial (~360 B/ns)
        # with ~650 ns descriptor-gen per DMA on the issuing queue, so the
        # startup is a sequencing problem: W first (small, feeds the stats
        # chain), then batch-0 x in fine row slices sized so the PE's chunk
        # consumption never outruns the x stream, then the bulk (weights
        # half 2, batch 1) which is needed much later.
        def xs(b, cc, r0, r1):
            r = slice(r0, r1)
            nc.sync.dma_start(out=xt[b][cc][:, r, :],
                              in_=x[b, cc * 128:(cc + 1) * 128, r, :])

        def issue_input_dmas():
            nc.sync.dma_start(out=wraw[0][:, 0:1, :], in_=w_okI[0:128, 0:1, :])
            nc.sync.dma_start(out=wraw[0][:, 1:2, :], in_=w_okI[0:128, 1:2, :])
            xs(0, 0, 0, 8)
            xs(0, 1, 0, 8)
            nc.sync.dma_start(out=wraw[0][:, 2:5, :], in_=w_okI[0:128, 2:5, :])
            xs(0, 0, 8, 16)
            xs(0, 1, 8, 16)
            nc.sync.dma_start(out=braw[0], in_=bias2d[0:128])
            for r0 in range(16, 64, 8):
                xs(0, 0, r0, r0 + 8)
                xs(0, 1, r0, r0 + 8)
            nc.sync.dma_start(out=wraw[1], in_=w_okI[128:256])
            nc.sync.dma_start(out=braw[1], in_=bias2d[128:256])
            for b in range(1, B_LOCAL):
                for cc in range(2):
                    for hh in range(2):
                        xs(b, cc, hh * 32, (hh + 1) * 32)

        # ---- weight standardization + PE transpose ----
        # wt[oc][:, k, ic, :] = [128 (in-sub), 128 (out-sub)] f32r lhsT tile
        wt = [cpool.tile([128, NTAP, 2, 128], f32r, name=f"wt_{oc}",
                         tag=f"wt_{oc}") for oc in range(2)]

        with tc.tile_pool(name="wprep", bufs=2) as wpool, \
             tc.tile_pool(name="tpsum", bufs=3, space="PSUM") as tpool, \
             tc.tile_pool(name="mmpsum", bufs=5, space="PSUM") as mpool, \
             tc.tile_pool(name="outp", bufs=OBUFS) as opool:

            def prewarm_pe(n):
                """f32 identity transposes (no data dependency beyond
                make_identity): busy the PE from ~t=0 so the HW clock-ramp
                window elapses before the real matmul stream begins."""
                for _ in range(n):
                    ptw = tpool.tile([128, 128], f32, name="ptw", tag="pt")
                    nc.tensor.transpose(ptw, ident_f32, ident_f32)

            def stats_a(oc, st, k):
                """ssq_k = sum(w_k^2) on ACT (Square + accum); sums_k on
                DVE. Independent of each other and of other taps."""
                ks = slice(k, k + 1)
                nc.scalar.activation(st["sqscratch"], wraw[oc][:, k, :],
                                     AF.Square, accum_out=st["ssq"][:, ks])
                nc.vector.reduce_sum(out=st["sums"][:, ks],
                                     in_=wraw[oc][:, k, :], axis=AX.X)

            def stats_b(oc, wn, st, k):
                """mu/var algebra on Pool (tiny [128,1] ops, with
                musums = (sums/IN_CH)*sums fused into one op), sqrt on
                ACT, reciprocal + fused normalize on DVE. mu is off the
                critical path (parallel to the var->inv chain)."""
                ks = slice(k, k + 1)
                nc.gpsimd.scalar_tensor_tensor(
                    out=st["musums"][:, ks], in0=st["sums"][:, ks],
                    scalar=1.0 / IN_CH, in1=st["sums"][:, ks],
                    op0=mybir.AluOpType.mult, op1=mybir.AluOpType.mult)
                nc.gpsimd.tensor_scalar_mul(st["mu"][:, ks],
                                            st["sums"][:, ks], 1.0 / IN_CH)
                nc.gpsimd.tensor_sub(out=st["var"][:, ks],
                                     in0=st["ssq"][:, ks],
                                     in1=st["musums"][:, ks])
                nc.scalar.sqrt(st["sd"][:, ks], st["var"][:, ks])
                nc.vector.reciprocal(st["inv"][:, ks], st["sd"][:, ks])
                # wn_k = (w_k - mu_k) * inv_k, one fused DVE op
                nc.vector.tensor_scalar(
                    out=wn[:, k, :], in0=wraw[oc][:, k, :],
                    scalar1=st["mu"][:, ks], scalar2=st["inv"][:, ks],
                    op0=mybir.AluOpType.subtract,
                    op1=mybir.AluOpType.mult)

            def transpose_tap(oc, wn, k):
                for ic in range(2):
                    pt = tpool.tile([128, 128], f32r, name="pt")
                    nc.tensor.transpose(
                        pt, wn[:, k, ic * 128:(ic + 1) * 128], ident)
                    # alternate PSUM->SBUF copy engine: DVE / ACT
                    if ic == 0:
                        nc.vector.tensor_copy(out=wt[oc][:, k, ic, :], in_=pt)
                    else:
                        nc.scalar.copy(wt[oc][:, k, ic, :], pt)

            def alloc_stats(suffix):
                st = {}
                for nm in ("ssq", "sums", "mu", "musums", "var", "sd", "inv"):
                    st[nm] = wpool.tile([128, NTAP], f32, name=f"{nm}{suffix}",
                                        tag=f"{nm}{suffix}")
                st["sqscratch"] = wpool.tile([128, IN_CH], f32,
                                             name=f"sqs{suffix}",
                                             tag=f"sqs{suffix}")
                return st

            def drain_chunk(b, oc, r0, nr, ps, split=False):
                ot = opool.tile([128, nr, OH], f32, name="ot", tag="ot")
                nc.scalar.activation(ot, ps, AF.Identity,
                                     bias=bnf[oc], scale=NF)
                # the last two chunks drain via the sync queue: it is idle
                # at the tail and its descriptor-gen is the cheapest, so the
                # final stores don't queue behind Pool's drain backlog
                eng = nc.sync if split else nc.gpsimd
                eng.dma_start(
                    out=y[b, oc * 128:(oc + 1) * 128, r0:r0 + nr, :],
                    in_=ot)

            def conv_chunk(b, oc, r0, split=False):
                nr = min(ROW_CHUNK, OH - r0)
                ps = mpool.tile([128, nr, OH], f32, name="ps", tag="ps")
                idx = 0
                for k, (kh, kw) in enumerate(KERNEL_KEYS):
                    for ic in range(2):
                        rhs = xt[b][ic][:, kh + r0:kh + r0 + nr, kw:kw + OH]
                        nc.tensor.matmul(ps, wt[oc][:, k, ic, :], rhs,
                                         start=(idx == 0), stop=(idx == 9))
                        idx += 1
                drain_chunk(b, oc, r0, nr, ps, split=split)

            def conv_group(b, oc, r0_start=0, split_last=False,
                           mid_hook=None):
                for r0 in range(r0_start, OH, ROW_CHUNK):
                    last2 = r0 + 2 * ROW_CHUNK >= OH
                    conv_chunk(b, oc, r0, split=(split_last and last2))
                    if mid_hook is not None and r0 + ROW_CHUNK > OH // 2:
                        mid_hook()
                        mid_hook = None

            # first two chunks of (b0, oc0): a 6-row chunk needing only x
            # rows 0-7 and an 8-row chunk needing rows 6-15, matching the
            # first two fine-grained x DMAs per cc.
            PREP0 = ((0, 6), (6, ROW_CHUNK))

            def prep0_fused():
                """oc0 prep with the first chunks' matmuls interleaved per
                tap, so PE work starts as soon as tap 0 is ready. Stats
                emission staggered one tap ahead so ACT's in-order queue
                never head-of-line blocks the next tap's Square."""
                st = alloc_stats("_0")
                wn = wpool.tile([128, NTAP, IN_CH], f32r, name="wn_0",
                                tag="wn_0")
                psf = [mpool.tile([128, nr, OH], f32, name=f"psf{c}",
                                  tag="ps")
                       for c, (r0, nr) in enumerate(PREP0)]
                stats_a(0, st, 0)
                stats_a(0, st, 1)
                for k, (kh, kw) in enumerate(KERNEL_KEYS):
                    stats_b(0, wn, st, k)
                    transpose_tap(0, wn, k)
                    for c, (r0, nr) in enumerate(PREP0):
                        for ic in range(2):
                            rhs = xt[0][ic][:, kh + r0:kh + r0 + nr,
                                            kw:kw + OH]
                            nc.tensor.matmul(
                                psf[c], wt[0][:, k, ic, :], rhs,
                                start=(k == 0 and ic == 0),
                                stop=(k == NTAP - 1 and ic == 1))
                    if k + 2 < NTAP:
                        stats_a(0, st, k + 2)
                for c, (r0, nr) in enumerate(PREP0):
                    drain_chunk(0, 0, r0, nr, psf[c])
                return PREP0[-1][0] + PREP0[-1][1]

            def prep(oc):
                st = alloc_stats(f"_{oc}")
                wn = wpool.tile([128, NTAP, IN_CH], f32r, name=f"wn_{oc}",
                                tag=f"wn_{oc}")
                stats_a(oc, st, 0)
                stats_a(oc, st, 1)
                for k in range(NTAP):
                    stats_b(oc, wn, st, k)
                    transpose_tap(oc, wn, k)
                    if k + 2 < NTAP:
                        stats_a(oc, st, k + 2)

            issue_input_dmas()
            prewarm_pe(PREWARM)
            # f32r identity for the real weight transposes; DVE is idle
            # during the prewarm so this never delays the stats chain.
            ident = cpool.tile([128, 128], f32r, name="ident")
            nc.vector.tensor_copy(out=ident, in_=ident_f32)
            # bias * NF on Pool so the ACT queue head stays free for stats
            for oc in range(2):
                nc.gpsimd.tensor_scalar_mul(bnf[oc], braw[oc], NF)
            r0_rest = prep0_fused()
            # oc1 prep emitted mid-group so its ACT stats ops enqueue ahead
            # of the later drain-activations (ACT's queue is in-order)
            conv_group(0, 0, r0_rest, mid_hook=lambda: prep(1))
            conv_group(0, 1)
            for b in range(1, B_LOCAL):
                conv_group(b, 0)
                conv_group(b, 1, split_last=(b == B_LOCAL - 1))


def _build_nc():
    import concourse.mybir as mybir
    import concourse.tile as tile
    from concourse import bacc

    f32 = mybir.dt.float32
    f32r = mybir.dt.float32r
    nc = bacc.Bacc("TRN2", target_bir_lowering=False, debug=False)
    x = nc.dram_tensor("x", (B_LOCAL, IN_CH, H, H), f32r,
                       kind="ExternalInput").ap()
    w = nc.dram_tensor("w", (len(KERNEL_KEYS), OUT_CH, IN_CH), f32,
                       kind="ExternalInput").ap()
    bias = nc.dram_tensor("bias", (OUT_CH,), f32, kind="ExternalInput").ap()
    y = nc.dram_tensor("y", (B_LOCAL, OUT_CH, OH, OH), f32,
                       kind="ExternalOutput").ap()

    with tile.TileContext(nc) as tc:
        _emit(tc, nc, y, x, w, bias)
    nc.compile()
    return nc


def _get_nc():
    global _compiled_nc
    if _compiled_nc is None:
        _compiled_nc = _build_nc()
    return _compiled_nc


def _make_in_maps(x, W, bias):
    x = np.ascontiguousarray(x, dtype=np.float32)
    W = np.ascontiguousarray(W, dtype=np.float32)
    bias = np.ascontiguousarray(bias, dtype=np.float32)
    return [
        {
            "x": np.ascontiguousarray(x[i * B_LOCAL:(i + 1) * B_LOCAL]),
            "w": W,
            "bias": bias,
        }
        for i in range(N_CORES)
    ]


def kernel(x, W, bias):
    from concourse import bass_utils

    nc = _get_nc()
    res = bass_utils.run_bass_kernel_spmd(
        nc, _make_in_maps(x, W, bias), core_ids=list(range(N_CORES)))
    return np.concatenate([r["y"] for r in res.results], axis=0)
